# revision 1
# baseline (speedup 1.0000x reference)
"""NT-Xent contrastive loss (SimCLR) on 8 Trainium2 NeuronCores.

Strategy (data-parallel, fully SPMD — no collectives):
  - z = concat(z_i, z_j) [8192, 1024], cast bf16 on host.
  - Each core c gets a ROTATED, transposed view of z (rows rolled by
    -c*1024): zt = z_rot.T [1024, 8192], kept fully resident in SBUF. The
    rotation puts each core's own 1024 rows at index 0, so the
    self-diagonal / positive-pair positions are the same compile-time
    constants on every core -> one NEFF for all 8 cores.
  - Norms are computed from the resident zt: DVE squares each k-tile
    chunk, PE reduces over the partition (k) axis with ones-matmuls whose
    STATIONARY operand is the squared chunk (output lands partition-major
    [128, 64]).  inv = 1/max(sqrt(sumsq), eps); a PE transpose + DRAM
    bounce + partition-broadcast DMA yields the column-wise table Bt.
  - Main loop over 64 [128 rows x 1024 cols] tiles: G = Z_own @ Z.T on PE
    (bf16, f32 psum accum); S = G * inv_r * inv_c in one DVE
    scalar_tensor_tensor into SBUF; self-diagonal masked with -100;
    exp(S/T) with fused per-row accumulation on ACT; positives extracted
    with an identity-masked multiply + fused accum; lse = ln(sumexp);
    partial = sum_rows(lse - pos) reduced to a scalar with a ones-matmul.
  - Host sums the 8 per-core partials and divides by 2N.

This container's walrus build only accepts ONE semaphore wait per
instruction (and none on CTRL-encoded ones like Drain), while Tile freely
emits several. Two workarounds below: the TileContext epilogue drain's waits
are re-emitted on DVE memsets, and a post-pass splits any multi-wait
instruction by inserting single-wait no-op "carrier" clones (per-engine
templates) just before it on the same engine stream.
"""

import copy

import numpy as np
import ml_dtypes


def _install_tile_drain_patch():
    import concourse.tile as tile
    from concourse import mybir
    from concourse.vector_clock import ScopedClock

    if getattr(tile.TileContext, "_drain_patch_installed", False):
        return

    def _drain_and_barrier(self, tick_clock, wait_clock):
        nc = self.nc
        drain_inst = nc.sync.drain()
        wait_clock.add_sem_waits(
            drain_inst.ins, ScopedClock({None: tick_clock.global_clock})
        )
        waits = list(drain_inst.ins.sync_info.on_wait)
        drain_inst.ins.sync_info.on_wait.clear()

        if waits:
            scr = nc.const_aps.tensor(0.0, (1, 1), mybir.dt.float32)
            for w in waits:
                ms = nc.vector.memset(scr, 0)
                if ms.ins.sync_info is None:
                    ms.ins.sync_info = mybir.SyncInfo(on_wait=[], on_update=[])
                ms.ins.sync_info.on_wait.append(w)

        nc.all_engine_barrier()
        assert self.sems is not None
        popped = nc._tile_sem_poison_stack.pop()
        assert popped is self._sem_poison
        nc.clear_and_free_semaphores(list(self.sems.allocated().values()))
        nc.all_engine_barrier()

    tile.TileContext._drain_and_barrier = _drain_and_barrier
    tile.TileContext._drain_patch_installed = True


_install_tile_drain_patch()

import concourse.bass as bass
import concourse.tile as tile
from concourse import mybir
from concourse.bass_utils import run_bass_kernel_spmd
from concourse.masks import make_identity

P = 128
D = 1024
R = 8192          # 2N rows
MY = 1024         # rows per core
KT = D // P       # 8 k-tiles
MT = MY // P      # 8 m-tiles
CW = 1024         # column chunk width
NCH = R // CW     # 8 chunks
CB = R // P       # 64 column blocks (norm layout)
TEMP = 0.07
BF16 = mybir.dt.bfloat16
F32 = mybir.dt.float32
AX = mybir.AxisListType
ALU = mybir.AluOpType
ACTF = mybir.ActivationFunctionType

TRACE = False          # set True externally (test harness) for NTFF profiling
LAST_RESULTS = None    # BassKernelResults of the last run (for the harness)

_NC_CACHE = None


def _split_multi_waits(nc, templates):
    """Rewrite any instruction carrying >1 sem waits: keep the last wait,
    move each extra onto a fresh single-wait clone of the same-engine no-op
    template inserted immediately before it (engine streams are in-order)."""
    n = 0
    for f in nc.m.functions:
        for bb in f.blocks:
            newlist = []
            for ins in bb.instructions:
                si = getattr(ins, "sync_info", None)
                if si is not None and si.on_wait and len(si.on_wait) > 1:
                    extras = list(si.on_wait[:-1])
                    keep = list(si.on_wait[-1:])
                    tmpl = templates.get(ins.engine)
                    assert tmpl is not None, (
                        f"no wait-carrier template for engine {ins.engine} "
                        f"({type(ins).__name__} {ins.name})"
                    )
                    for w in extras:
                        c = copy.deepcopy(tmpl)
                        c.name = f"wcarrier-{n}"
                        n += 1
                        c.sync_info = mybir.SyncInfo(on_wait=[w], on_update=[])
                        newlist.append(c)
                    del si.on_wait[:]
                    si.on_wait.extend(keep)
                newlist.append(ins)
            bb.instructions[:] = newlist
    return n


def build():
    nc = bass.Bass()
    zt = nc.dram_tensor("zt", [D, R], BF16, kind="ExternalInput")
    out = nc.dram_tensor("partial", [1, 1], F32, kind="ExternalOutput")

    templates = {}

    with tile.TileContext(nc) as tc:
        with (
            tc.tile_pool(name="singles", bufs=1) as singles,
            tc.tile_pool(name="work", bufs=3) as work,
            tc.tile_pool(name="sbuf_s", bufs=3) as sbuf_s,
            tc.tile_pool(name="psum_g", bufs=3, space="PSUM") as psum_g,
            tc.tile_pool(name="psum_m", bufs=1, space="PSUM") as psum_m,
            tc.tile_pool(name="dram", bufs=1, space="DRAM") as dram,
        ):
            zt_k = [singles.tile([P, R], BF16, name=f"zt{k}") for k in range(KT)]
            Bt = singles.tile([P, R], BF16)            # inv-norm bcast by col
            I128 = singles.tile([P, P], F32)
            ones = singles.tile([P, 1], F32)
            ones_bf = singles.tile([P, 1], BF16)
            sumsq = singles.tile([P, CB], F32)
            normt = singles.tile([P, CB], F32)
            invt = singles.tile([P, CB], F32)
            slots = singles.tile([P, MT * NCH], F32)
            post = singles.tile([P, MT], F32)
            sumexp = singles.tile([P, MT], F32)
            lse = singles.tile([P, MT], F32)
            contribs = singles.tile([P, MT], F32)
            csum = singles.tile([P, 1], F32)
            out_sb = singles.tile([1, 1], F32)
            junk_exp = singles.tile([P, CW], F32)
            junk_pos = singles.tile([P, P], F32)
            trs = singles.tile([CB, P], F32)
            # wait-carrier scratches (one per engine, never read)
            scr_v = singles.tile([1, 1], F32)
            scr_a = singles.tile([1, 1], F32)
            scr_p = singles.tile([1, 1], F32)
            # shared misc PSUM bank: norm accum [128,64] / transpose [64,128]
            # / final [1,1] — used at disjoint times (Tile serializes).
            misc_ps = psum_m.tile([P, P], F32)
            dummy_ps = psum_m.tile([1, 1], F32)
            inv_dram = dram.tile([CB, P], F32)

            # --- wait-carrier templates (harmless one-off ops) ---
            c0 = nc.const_aps.tensor(0.0, (1, 1), F32)
            templates[mybir.EngineType.DVE] = nc.vector.memset(scr_v[:], 0).ins
            templates[mybir.EngineType.Activation] = nc.scalar.copy(
                scr_a[:], c0).ins
            templates[mybir.EngineType.Pool] = nc.gpsimd.memset(scr_p[:], 0).ins
            templates[mybir.EngineType.PE] = nc.tensor.matmul(
                dummy_ps[:], c0, c0, start=True, stop=True,
                skip_group_check=True).ins

            make_identity(nc, I128[:, :])
            nc.vector.memset(ones[:], 1.0)
            nc.vector.memset(ones_bf[:], 1.0)

            # Load resident Z^T (8 x 2 MB) and fold the norm reduction into
            # the stream: square each arriving chunk on DVE, reduce over the
            # k (partition) axis on PE with the squared chunk stationary.
            ss = misc_ps[:, 0:CB]                      # [128, 64] f32 accum
            for k in range(KT):
                nc.gpsimd.dma_start(out=zt_k[k][:], in_=zt[k * P:(k + 1) * P, :])
                for cc in range(R // CW):              # 8 x 1024-col chunks
                    sq = work.tile([P, CW], BF16, tag="sq")
                    nc.vector.tensor_mul(
                        sq[:], zt_k[k][:, cc * CW:(cc + 1) * CW],
                        zt_k[k][:, cc * CW:(cc + 1) * CW])
                    for cb in range(CW // P):          # 8 x 128-col blocks
                        g = cc * (CW // P) + cb
                        nc.tensor.matmul(
                            ss[:, g:g + 1],
                            sq[:, cb * P:(cb + 1) * P],
                            ones_bf[:],
                            start=(k == 0), stop=(k == KT - 1),
                            skip_group_check=True)
            nc.vector.tensor_copy(sumsq[:], ss)
            nc.scalar.sqrt(normt[:], sumsq[:])
            nc.vector.tensor_scalar_max(normt[:], normt[:], 1e-8)
            nc.vector.reciprocal(invt[:], normt[:])

            # Column-broadcast inv table: PE-transpose -> DRAM bounce ->
            # partition-broadcast DMA (bf16).
            trp = misc_ps[0:CB, :]                     # [64, 128] view
            nc.tensor.transpose(trp, invt[:], I128[:])
            nc.vector.tensor_copy(trs[:], trp)
            nc.gpsimd.dma_start(out=inv_dram[:], in_=trs[:])
            src = inv_dram[:]
            bcast = bass.AP(tensor=src.tensor, offset=src.offset,
                            ap=[[0, P], [1, R]])
            nc.gpsimd.dma_start(out=Bt[:], in_=bcast)

            inv_t = float(1.0 / TEMP)
            for j in range(NCH):
                for m in range(MT):
                    g = psum_g.tile([P, CW], F32)
                    for k in range(KT):
                        for h in range(CW // 512):  # N<=512 per matmul
                            nc.tensor.matmul(
                                g[:, h * 512:(h + 1) * 512],
                                zt_k[k][:, m * P:(m + 1) * P],
                                zt_k[k][:, j * CW + h * 512:j * CW + (h + 1) * 512],
                                start=(k == 0), stop=(k == KT - 1),
                                skip_group_check=True)
                    s = sbuf_s.tile([P, CW], F32)
                    nc.vector.scalar_tensor_tensor(
                        out=s[:], in0=g[:], scalar=invt[:, m:m + 1],
                        in1=Bt[:, j * CW:(j + 1) * CW],
                        op0=ALU.mult, op1=ALU.mult)
                    if j == 0:
                        # self-similarity diagonal -> -inf (via -100 pre /T)
                        off = m * P
                        nc.vector.scalar_tensor_tensor(
                            out=s[:, off:off + P], in0=I128[:], scalar=-100.0,
                            in1=s[:, off:off + P], op0=ALU.mult, op1=ALU.add)
                    if j == 4096 // CW:
                        # positive pair: rotated column = row + 4096.
                        # post[m] = sum(S_slice * I) (pre-1/T; folded later)
                        off = m * P
                        nc.vector.scalar_tensor_tensor(
                            out=junk_pos[:], in0=s[:, off:off + P], scalar=1.0,
                            in1=I128[:], op0=ALU.mult, op1=ALU.mult,
                            accum_out=post[:, m:m + 1])
                    nc.scalar.activation(
                        out=junk_exp[:], in_=s[:], func=ACTF.Exp,
                        scale=inv_t,
                        accum_out=slots[:, m * NCH + j:m * NCH + j + 1])

            for m in range(MT):
                nc.vector.reduce_sum(
                    out=sumexp[:, m:m + 1],
                    in_=slots[:, m * NCH:(m + 1) * NCH], axis=AX.X)
            nc.scalar.activation(out=lse[:], in_=sumexp[:], func=ACTF.Ln)
            # contribs = lse - post/T  ==  (post * -1/T) + lse
            nc.vector.scalar_tensor_tensor(
                out=contribs[:], in0=post[:], scalar=-inv_t,
                in1=lse[:], op0=ALU.mult, op1=ALU.add)
            nc.vector.reduce_sum(out=csum[:], in_=contribs[:], axis=AX.X)
            fin = misc_ps[0:1, 0:1]
            nc.tensor.matmul(fin, ones[:], csum[:], start=True, stop=True,
                             skip_group_check=True)
            nc.vector.tensor_copy(out_sb[:], fin)
            nc.gpsimd.dma_start(out=out[:], in_=out_sb[:])

    _split_multi_waits(nc, templates)
    return nc


def kernel(z_i: np.ndarray, z_j: np.ndarray) -> np.ndarray:
    global _NC_CACHE, LAST_RESULTS
    z = np.concatenate([np.asarray(z_i), np.asarray(z_j)], axis=0)
    zb = z.astype(ml_dtypes.bfloat16)

    in_maps = []
    for c in range(8):
        zrot = np.roll(zb, -c * MY, axis=0)
        in_maps.append({"zt": np.ascontiguousarray(zrot.T)})

    if _NC_CACHE is None:
        _NC_CACHE = build()

    res = run_bass_kernel_spmd(
        _NC_CACHE, in_maps, core_ids=list(range(8)), trace=TRACE)
    LAST_RESULTS = res

    total = 0.0
    for c in range(8):
        total += float(res.results[c]["partial"][0, 0])
    return np.float32(total / R)



# revision 3
# speedup vs baseline: 2.1949x; 2.1949x over previous
"""NT-Xent contrastive loss (SimCLR) on 8 Trainium2 NeuronCores.

Strategy (data-parallel, fully SPMD — no collectives):
  - Host: z = concat(z_i, z_j) [8192, 1024], L2-normalize rows in f64,
    scale by S=256 and quantize to fp8 e4m3. Each core c gets a ROTATED,
    transposed view (rows rolled by -c*1024): zt = zn_rot.T [1024, 8192].
    The rotation puts each core's own 1024 rows at index 0, so the
    self-diagonal / positive-pair positions are the same compile-time
    constants on every core -> one NEFF for all 8 cores.
  - Device: zt resident in SBUF as 4 k-pair tiles [128, 2, 8192] fp8
    (k = t*256 + i*128 + p), streamed in by column slabs so compute
    starts after the first slab.
  - Main loop over (jj, m): G = Z_own[m] @ Z[:, jj-chunk] via fp8
    DoubleRow matmuls (K=256 per instruction, N=512, 4-step PSUM
    accumulation).  G holds S^2 * cos similarities.  Self-diagonal
    masked by adding -2^24 * I on DVE; positive pair (rotated column =
    row + 4096) extracted with an identity-masked multiply + fused
    accum; exp(G * invT/S^2) with fused per-row accumulation on ACT.
  - lse = ln(sum exp); partial = sum_rows(lse - pos*invT/S^2) reduced
    to a scalar with a ones-matmul.  Host sums the 8 per-core partials
    and divides by 2N.

This container's walrus build only accepts ONE semaphore wait per
instruction (and none on CTRL-encoded ones like Drain), while Tile freely
emits several. Two workarounds below: the TileContext epilogue drain's waits
are re-emitted on DVE memsets, and a post-pass splits any multi-wait
instruction by inserting single-wait no-op "carrier" clones (per-engine
templates) just before it on the same engine stream.
"""

import copy

import numpy as np
import ml_dtypes


def _install_tile_drain_patch():
    import concourse.tile as tile
    from concourse import mybir
    from concourse.vector_clock import ScopedClock

    if getattr(tile.TileContext, "_drain_patch_installed", False):
        return

    def _drain_and_barrier(self, tick_clock, wait_clock):
        nc = self.nc
        drain_inst = nc.sync.drain()
        wait_clock.add_sem_waits(
            drain_inst.ins, ScopedClock({None: tick_clock.global_clock})
        )
        waits = list(drain_inst.ins.sync_info.on_wait)
        drain_inst.ins.sync_info.on_wait.clear()

        if waits:
            scr = nc.const_aps.tensor(0.0, (1, 1), mybir.dt.float32)
            for w in waits:
                ms = nc.vector.memset(scr, 0)
                if ms.ins.sync_info is None:
                    ms.ins.sync_info = mybir.SyncInfo(on_wait=[], on_update=[])
                ms.ins.sync_info.on_wait.append(w)

        nc.all_engine_barrier()
        assert self.sems is not None
        popped = nc._tile_sem_poison_stack.pop()
        assert popped is self._sem_poison
        nc.clear_and_free_semaphores(list(self.sems.allocated().values()))
        nc.all_engine_barrier()

    tile.TileContext._drain_and_barrier = _drain_and_barrier
    tile.TileContext._drain_patch_installed = True


_install_tile_drain_patch()

import concourse.bass as bass
import concourse.tile as tile
from concourse import mybir
from concourse.bass_utils import run_bass_kernel_spmd
from concourse.masks import make_identity

P = 128
D = 1024          # feature dim (contraction K)
R = 8192          # 2N rows
MY = 1024         # rows per core
TP = 4            # k-pair tiles: K = 1024 = 4 * 256
MT = MY // P      # 8 m-tiles
CW = 1024         # column chunk width (ACT exp tile)
NCH = R // CW     # 8 chunks
NSUB = CW // 512  # 2 psum 512-col banks per chunk
TEMP = 0.07
FSCALE = 256.0    # host fp8 scale: sim in psum = FSCALE^2 * cos
F8 = mybir.dt.float8e4
F32 = mybir.dt.float32
AX = mybir.AxisListType
ALU = mybir.AluOpType
ACTF = mybir.ActivationFunctionType
DR = mybir.MatmulPerfMode.DoubleRow

TRACE = False          # set True externally (test harness) for NTFF profiling
LAST_RESULTS = None    # BassKernelResults of the last run (for the harness)

_NC_CACHE = None


def _split_multi_waits(nc, templates):
    """Rewrite any instruction carrying >1 sem waits: keep the last wait,
    move each extra onto a fresh single-wait clone of the same-engine no-op
    template inserted immediately before it (engine streams are in-order)."""
    n = 0
    for f in nc.m.functions:
        for bb in f.blocks:
            newlist = []
            for ins in bb.instructions:
                si = getattr(ins, "sync_info", None)
                if si is not None and si.on_wait and len(si.on_wait) > 1:
                    extras = list(si.on_wait[:-1])
                    keep = list(si.on_wait[-1:])
                    tmpl = templates.get(ins.engine)
                    assert tmpl is not None, (
                        f"no wait-carrier template for engine {ins.engine} "
                        f"({type(ins).__name__} {ins.name})"
                    )
                    for w in extras:
                        c = copy.deepcopy(tmpl)
                        c.name = f"wcarrier-{n}"
                        n += 1
                        c.sync_info = mybir.SyncInfo(on_wait=[w], on_update=[])
                        newlist.append(c)
                    del si.on_wait[:]
                    si.on_wait.extend(keep)
                newlist.append(ins)
            bb.instructions[:] = newlist
    return n


def build():
    nc = bass.Bass()
    zt = nc.dram_tensor("zt", [D, R], F8, kind="ExternalInput")
    out = nc.dram_tensor("partial", [1, 1], F32, kind="ExternalOutput")

    templates = {}
    inv_t = float(1.0 / TEMP)
    exp_scale = float(inv_t / (FSCALE * FSCALE))

    with tile.TileContext(nc) as tc:
        with (
            tc.tile_pool(name="singles", bufs=1) as singles,
            tc.tile_pool(name="psum_g", bufs=3, space="PSUM") as psum_g,
            tc.tile_pool(name="psum_m", bufs=1, space="PSUM") as psum_m,
        ):
            # resident fp8 Z^T as 4 k-pair tiles: [p, i, col], k = t*256+i*128+p
            ztp = [singles.tile([P, 2, R], F8, name=f"ztp{t}") for t in range(TP)]
            I128 = singles.tile([P, P], F32)
            ones = singles.tile([P, 1], F32)
            slots = singles.tile([P, MT * NCH], F32)
            post = singles.tile([P, MT], F32)
            sumexp = singles.tile([P, MT], F32)
            lse = singles.tile([P, MT], F32)
            contribs = singles.tile([P, MT], F32)
            csum = singles.tile([P, 1], F32)
            out_sb = singles.tile([1, 1], F32)
            junk_exp = singles.tile([P, CW], F32)
            junk_pos = singles.tile([P, P], F32)
            # wait-carrier scratches (one per engine, never read)
            scr_v = singles.tile([1, 1], F32)
            scr_a = singles.tile([1, 1], F32)
            scr_p = singles.tile([1, 1], F32)
            fin_ps = psum_m.tile([1, 1], F32)
            dummy_ps = psum_m.tile([1, 1], F32)

            # --- wait-carrier templates (harmless one-off ops) ---
            c0 = nc.const_aps.tensor(0.0, (1, 1), F32)
            templates[mybir.EngineType.DVE] = nc.vector.memset(scr_v[:], 0).ins
            templates[mybir.EngineType.Activation] = nc.scalar.copy(
                scr_a[:], c0).ins
            templates[mybir.EngineType.Pool] = nc.gpsimd.memset(scr_p[:], 0).ins
            templates[mybir.EngineType.PE] = nc.tensor.matmul(
                dummy_ps[:], c0, c0, start=True, stop=True,
                skip_group_check=True).ins

            make_identity(nc, I128[:, :])
            nc.vector.memset(ones[:], 1.0)

            # Stream resident Z^T by column slabs (all k for a column range
            # arrives together so compute on slab jj can start immediately).
            zt_base = zt[0:1, 0:1]
            for jj in range(NCH):
                for t in range(TP):
                    src = bass.AP(
                        tensor=zt_base.tensor,
                        offset=t * 256 * R + jj * CW,
                        ap=[[R, P], [P * R, 2], [1, CW]])
                    nc.gpsimd.dma_start(
                        out=ztp[t][:, :, jj * CW:(jj + 1) * CW], in_=src)

            for jj in range(NCH):
                for m in range(MT):
                    g = psum_g.tile([P, CW], F32, tag="g")
                    for n in range(NSUB):
                        for t in range(TP):
                            nc.tensor.matmul(
                                g[:, n * 512:(n + 1) * 512],
                                ztp[t][:, :, m * P:(m + 1) * P],
                                ztp[t][:, :, jj * CW + n * 512:
                                       jj * CW + (n + 1) * 512],
                                start=(t == 0), stop=(t == TP - 1),
                                perf_mode=DR, skip_group_check=True)
                    off = m * P
                    if jj == 0:
                        # self-similarity diagonal -> -inf (big negative)
                        nc.vector.scalar_tensor_tensor(
                            out=g[:, off:off + P], in0=I128[:],
                            scalar=-16777216.0, in1=g[:, off:off + P],
                            op0=ALU.mult, op1=ALU.add)
                    if jj == 4096 // CW:
                        # positive pair: rotated column = row + 4096.
                        # post[m] = sum(G_slice * I)  (= FSCALE^2 * cos_pos)
                        nc.vector.scalar_tensor_tensor(
                            out=junk_pos[:], in0=g[:, off:off + P], scalar=1.0,
                            in1=I128[:], op0=ALU.mult, op1=ALU.mult,
                            accum_out=post[:, m:m + 1])
                    nc.scalar.activation(
                        out=junk_exp[:], in_=g[:], func=ACTF.Exp,
                        scale=exp_scale,
                        accum_out=slots[:, m * NCH + jj:m * NCH + jj + 1])

            for m in range(MT):
                nc.vector.reduce_sum(
                    out=sumexp[:, m:m + 1],
                    in_=slots[:, m * NCH:(m + 1) * NCH], axis=AX.X)
            nc.scalar.activation(out=lse[:], in_=sumexp[:], func=ACTF.Ln)
            # contribs = lse - pos*invT/S^2  ==  (post * -exp_scale) + lse
            nc.vector.scalar_tensor_tensor(
                out=contribs[:], in0=post[:], scalar=-exp_scale,
                in1=lse[:], op0=ALU.mult, op1=ALU.add)
            nc.vector.reduce_sum(out=csum[:], in_=contribs[:], axis=AX.X)
            nc.tensor.matmul(fin_ps[:], ones[:], csum[:], start=True,
                             stop=True, skip_group_check=True)
            nc.vector.tensor_copy(out_sb[:], fin_ps[:])
            nc.gpsimd.dma_start(out=out[:], in_=out_sb[:])

    _split_multi_waits(nc, templates)
    return nc


def kernel(z_i: np.ndarray, z_j: np.ndarray) -> np.ndarray:
    global _NC_CACHE, LAST_RESULTS
    z = np.concatenate([np.asarray(z_i), np.asarray(z_j)], axis=0)
    z = z.astype(np.float64)
    nrm = np.maximum(np.sqrt((z * z).sum(axis=1, keepdims=True)), 1e-8)
    zn = ((z / nrm) * FSCALE).astype(np.float32)

    in_maps = []
    for c in range(8):
        zrot = np.roll(zn, -c * MY, axis=0)
        in_maps.append(
            {"zt": np.ascontiguousarray(zrot.T).astype(ml_dtypes.float8_e4m3)})

    if _NC_CACHE is None:
        _NC_CACHE = build()

    res = run_bass_kernel_spmd(
        _NC_CACHE, in_maps, core_ids=list(range(8)), trace=TRACE)
    LAST_RESULTS = res

    total = 0.0
    for c in range(8):
        total += float(res.results[c]["partial"][0, 0])
    return np.float32(total / R)


# revision 6
# speedup vs baseline: 2.4122x; 1.0990x over previous
"""NT-Xent contrastive loss (SimCLR) on 8 Trainium2 NeuronCores.

Strategy (data-parallel, fully SPMD — no collectives):
  - Host: z = concat(z_i, z_j) [8192, 1024], L2-normalize rows in f64,
    scale by S=256 and quantize to fp8 e4m3. Each core c gets a ROTATED,
    transposed view (rows rolled by -c*1024): zt = zn_rot.T [1024, 8192].
    The rotation puts each core's own 1024 rows at index 0, so the
    self-diagonal / positive-pair positions are the same compile-time
    constants on every core -> one NEFF for all 8 cores.
  - Device: zt resident in SBUF as 4 k-pair tiles [128, 2, 8192] fp8
    (k = t*256 + i*128 + p), streamed in by column slabs so compute
    starts after the first slab.
  - Main loop over (jj, m): G = Z_own[m] @ Z[:, jj-chunk] via fp8
    DoubleRow matmuls (K=256 per instruction, N=512, 4-step PSUM
    accumulation).  G holds S^2 * cos similarities.  exp(G * invT/S^2)
    with fused per-row accumulation on ACT; ACT is the ONLY psum reader
    so every PSUM WAR is a single semaphore.
  - Self-similarity is NOT masked in psum: its exp lands in the row
    accumulator and the exact same f32 value is extracted from the exp'd
    SBUF tile (identity-masked DVE accum) and subtracted at the tail.
    The positive pair (rotated column = row + 4096) is also extracted
    from the exp'd tile and recovered linearly via Ln at the tail.
  - lse = ln(sum exp - exp_diag); partial = sum_rows(lse - ln(exp_pos))
    reduced to a scalar with a ones-matmul.  Host sums the 8 per-core
    partials and divides by 2N.

This container's walrus build only accepts ONE semaphore wait per
instruction (and none on CTRL-encoded ones like Drain), while Tile freely
emits several. Two workarounds below: the TileContext epilogue drain's waits
are re-emitted on DVE memsets, and a post-pass splits any multi-wait
instruction by inserting single-wait no-op "carrier" clones (per-engine
templates) just before it on the same engine stream.
"""

import copy

import numpy as np
import ml_dtypes


def _install_tile_drain_patch():
    import concourse.tile as tile
    from concourse import mybir
    from concourse.vector_clock import ScopedClock

    if getattr(tile.TileContext, "_drain_patch_installed", False):
        return

    def _drain_and_barrier(self, tick_clock, wait_clock):
        nc = self.nc
        drain_inst = nc.sync.drain()
        wait_clock.add_sem_waits(
            drain_inst.ins, ScopedClock({None: tick_clock.global_clock})
        )
        waits = list(drain_inst.ins.sync_info.on_wait)
        drain_inst.ins.sync_info.on_wait.clear()

        if waits:
            scr = nc.const_aps.tensor(0.0, (1, 1), mybir.dt.float32)
            for w in waits:
                ms = nc.vector.memset(scr, 0)
                if ms.ins.sync_info is None:
                    ms.ins.sync_info = mybir.SyncInfo(on_wait=[], on_update=[])
                ms.ins.sync_info.on_wait.append(w)

        nc.all_engine_barrier()
        assert self.sems is not None
        popped = nc._tile_sem_poison_stack.pop()
        assert popped is self._sem_poison
        nc.clear_and_free_semaphores(list(self.sems.allocated().values()))
        nc.all_engine_barrier()

    tile.TileContext._drain_and_barrier = _drain_and_barrier
    tile.TileContext._drain_patch_installed = True


_install_tile_drain_patch()

import concourse.bass as bass
import concourse.tile as tile
from concourse import mybir
from concourse.bass_utils import run_bass_kernel_spmd
from concourse.masks import make_identity

P = 128
D = 1024          # feature dim (contraction K)
R = 8192          # 2N rows
MY = 1024         # rows per core
TP = 4            # k-pair tiles: K = 1024 = 4 * 256
MT = MY // P      # 8 m-tiles
CW = 2048         # column chunk width (ACT exp tile, 4 psum banks)
NCH = R // CW     # 4 chunks
NSUB = CW // 512  # 4 psum 512-col banks per chunk
TEMP = 0.07
FSCALE = 256.0    # host fp8 scale: sim in psum = FSCALE^2 * cos
F8 = mybir.dt.float8e4
BF16 = mybir.dt.bfloat16
F32 = mybir.dt.float32
AX = mybir.AxisListType
ALU = mybir.AluOpType
ACTF = mybir.ActivationFunctionType
DR = mybir.MatmulPerfMode.DoubleRow

TRACE = False          # set True externally (test harness) for NTFF profiling
LAST_RESULTS = None    # BassKernelResults of the last run (for the harness)

_NC_CACHE = None


def _split_multi_waits(nc, templates):
    """Rewrite any instruction carrying >1 sem waits: keep the last wait,
    move each extra onto a fresh single-wait clone of the same-engine no-op
    template inserted immediately before it (engine streams are in-order)."""
    n = 0
    for f in nc.m.functions:
        for bb in f.blocks:
            newlist = []
            for ins in bb.instructions:
                si = getattr(ins, "sync_info", None)
                if si is not None and si.on_wait and len(si.on_wait) > 1:
                    extras = list(si.on_wait[:-1])
                    keep = list(si.on_wait[-1:])
                    tmpl = templates.get(ins.engine)
                    assert tmpl is not None, (
                        f"no wait-carrier template for engine {ins.engine} "
                        f"({type(ins).__name__} {ins.name})"
                    )
                    for w in extras:
                        c = copy.deepcopy(tmpl)
                        c.name = f"wcarrier-{n}"
                        n += 1
                        c.sync_info = mybir.SyncInfo(on_wait=[w], on_update=[])
                        newlist.append(c)
                    del si.on_wait[:]
                    si.on_wait.extend(keep)
                newlist.append(ins)
            bb.instructions[:] = newlist
    return n


def build():
    nc = bass.Bass()
    zt = nc.dram_tensor("zt", [D, R], F8, kind="ExternalInput")
    out = nc.dram_tensor("partial", [1, 1], F32, kind="ExternalOutput")

    templates = {}
    inv_t = float(1.0 / TEMP)
    exp_scale = float(inv_t / (FSCALE * FSCALE))

    with tile.TileContext(nc) as tc:
        with (
            tc.tile_pool(name="singles", bufs=1) as singles,
            tc.tile_pool(name="junk", bufs=3) as junkp,
            tc.tile_pool(name="psum_g", bufs=2, space="PSUM") as psum_g,
        ):
            # resident fp8 Z^T as 4 k-pair tiles: [p, i, col], k = t*256+i*128+p
            ztp = [singles.tile([P, 2, R], F8, name=f"ztp{t}") for t in range(TP)]
            I128 = singles.tile([P, P], F32)
            ones = singles.tile([P, 1], F32)
            slots = singles.tile([P, MT * NCH], F32)
            dexpv = singles.tile([P, MT], F32)
            posexp = singles.tile([P, MT], F32)
            sumexp = singles.tile([P, MT], F32)
            sumcor = singles.tile([P, MT], F32)
            lse = singles.tile([P, MT], F32)
            lpos = singles.tile([P, MT], F32)
            contribs = singles.tile([P, MT], F32)
            csum = singles.tile([P, 1], F32)
            out_sb = singles.tile([1, 1], F32)
            junk_ext = singles.tile([P, P], F32)
            # wait-carrier scratches (one per engine, never read)
            scr_v = singles.tile([1, 1], F32)
            scr_a = singles.tile([1, 1], F32)
            scr_p = singles.tile([1, 1], F32)
            scr_w = singles.tile([1, 1], BF16)

            # --- wait-carrier templates (harmless one-off ops) ---
            c0 = nc.const_aps.tensor(0.0, (1, 1), F32)
            templates[mybir.EngineType.DVE] = nc.vector.memset(scr_v[:], 0).ins
            templates[mybir.EngineType.Activation] = nc.scalar.copy(
                scr_a[:], c0).ins
            templates[mybir.EngineType.Pool] = nc.gpsimd.memset(scr_p[:], 0).ins
            nc.vector.memset(scr_w[:], 0.0)
            templates[mybir.EngineType.PE] = nc.tensor.ldweights(scr_w[:]).ins

            make_identity(nc, I128[:, :])
            nc.vector.memset(ones[:], 1.0)

            # Stream resident Z^T by column slabs (all k for a column range
            # arrives together so compute on slab jj can start immediately).
            # First slab (own block, also all matmul weights) split across 4
            # engine sequencers for minimum latency-to-first-matmul.
            zt_base = zt[0:1, 0:1]
            dma_engines = [nc.sync, nc.scalar, nc.gpsimd]

            def slab_dma(c0_, c1_, engines):
                for t in range(TP):
                    src = bass.AP(
                        tensor=zt_base.tensor,
                        offset=t * 256 * R + c0_,
                        ap=[[R, P], [P * R, 2], [1, c1_ - c0_]])
                    engines[t % len(engines)].dma_start(
                        out=ztp[t][:, :, c0_:c1_], in_=src)

            slab_dma(0, 1024, dma_engines)
            slab_dma(1024, 2048, [nc.sync, nc.gpsimd])
            for s in range(1, NCH):
                slab_dma(s * CW, (s + 1) * CW, [nc.sync, nc.gpsimd])

            for jj in range(NCH):
                for m in range(MT):
                    g = psum_g.tile([P, CW], F32, tag="g")
                    for n in range(NSUB):
                        for t in range(TP):
                            nc.tensor.matmul(
                                g[:, n * 512:(n + 1) * 512],
                                ztp[t][:, :, m * P:(m + 1) * P],
                                ztp[t][:, :, jj * CW + n * 512:
                                       jj * CW + (n + 1) * 512],
                                start=(t == 0), stop=(t == TP - 1),
                                perf_mode=DR, skip_group_check=True)
                    ju = junkp.tile([P, CW], F32, tag="ju")
                    nc.scalar.activation(
                        out=ju[:], in_=g[:], func=ACTF.Exp,
                        scale=exp_scale,
                        accum_out=slots[:, m * NCH + jj:m * NCH + jj + 1])
                    off = m * P
                    if jj == 0:
                        # exp of self-similarity: extract to subtract at tail
                        nc.vector.scalar_tensor_tensor(
                            out=junk_ext[:], in0=ju[:, off:off + P], scalar=1.0,
                            in1=I128[:], op0=ALU.mult, op1=ALU.mult,
                            accum_out=dexpv[:, m:m + 1])
                    if jj == 4096 // CW:
                        # positive pair (rotated column = row + 4096):
                        # extract exp, recover linearly via Ln at tail
                        nc.vector.scalar_tensor_tensor(
                            out=junk_ext[:], in0=ju[:, off:off + P], scalar=1.0,
                            in1=I128[:], op0=ALU.mult, op1=ALU.mult,
                            accum_out=posexp[:, m:m + 1])

            for m in range(MT):
                nc.vector.reduce_sum(
                    out=sumexp[:, m:m + 1],
                    in_=slots[:, m * NCH:(m + 1) * NCH], axis=AX.X)
            # sumcor = sumexp - exp(diag); lse = ln(sumcor)
            nc.vector.scalar_tensor_tensor(
                out=sumcor[:], in0=dexpv[:], scalar=-1.0,
                in1=sumexp[:], op0=ALU.mult, op1=ALU.add)
            nc.scalar.activation(out=lse[:], in_=sumcor[:], func=ACTF.Ln)
            nc.scalar.activation(out=lpos[:], in_=posexp[:], func=ACTF.Ln)
            # contribs = lse - ln(exp(pos*scale)) = lse - pos*invT
            nc.vector.scalar_tensor_tensor(
                out=contribs[:], in0=lpos[:], scalar=-1.0,
                in1=lse[:], op0=ALU.mult, op1=ALU.add)
            nc.vector.reduce_sum(out=csum[:], in_=contribs[:], axis=AX.X)
            ftile = psum_g.tile([P, CW], F32, tag="g")
            fin = ftile[0:1, 0:1]
            nc.tensor.matmul(fin, ones[:], csum[:], start=True,
                             stop=True, skip_group_check=True)
            nc.vector.tensor_copy(out_sb[:], fin)
            nc.gpsimd.dma_start(out=out[:], in_=out_sb[:])

    _split_multi_waits(nc, templates)
    return nc


def kernel(z_i: np.ndarray, z_j: np.ndarray) -> np.ndarray:
    global _NC_CACHE, LAST_RESULTS
    z = np.concatenate([np.asarray(z_i), np.asarray(z_j)], axis=0)
    z = z.astype(np.float64)
    nrm = np.maximum(np.sqrt((z * z).sum(axis=1, keepdims=True)), 1e-8)
    zn = ((z / nrm) * FSCALE).astype(np.float32)

    in_maps = []
    for c in range(8):
        zrot = np.roll(zn, -c * MY, axis=0)
        in_maps.append(
            {"zt": np.ascontiguousarray(zrot.T).astype(ml_dtypes.float8_e4m3)})

    if _NC_CACHE is None:
        _NC_CACHE = build()

    res = run_bass_kernel_spmd(
        _NC_CACHE, in_maps, core_ids=list(range(8)), trace=TRACE)
    LAST_RESULTS = res

    total = 0.0
    for c in range(8):
        total += float(res.results[c]["partial"][0, 0])
    return np.float32(total / R)
